# revision 19
# baseline (speedup 1.0000x reference)
"""3-branch 2-layer GAT classifier on 8 Trainium2 NeuronCores (Bass/Tile).

Strategy (edge-cut sharding per the hint):
- Nodes (and their incoming edges) are sharded contiguously across the 8
  cores; each core owns N/8 destination nodes for both GAT layers.
- Layer 1 is gather-free: the host expands x rows into per-edge-slot order
  (integer indexing only) as per-tile contiguous blocks holding all three
  branches; the tensor engine computes each slot's [feat | el] row via
  per-slot matmuls against [W | W.al | W.ar]. A trailing indicator row in
  the expanded input turns padding slots' el into -1e30 so they drop out
  of the edge softmax. The big streams are issued round-robin across the
  sync/scalar/gpsimd DMA queues so descriptors spread over the 16 DMA
  engines.
- The edge softmax is batched: one leaky-relu / exp / sum / reciprocal
  instruction per tile covers all 3 branches x 2 heads; exp runs without
  the max trick (logits are O(1)); the weighted aggregation multiplies
  the attention into the slot-feature tile in place and reduces over
  slots with one strided reduce per branch.
- Layer 2: each core computes its shard of the layer-2 node table (rows
  [feat2|el2] per branch, 320 wide) from its aggregated h1 rows, shards
  are AllGathered (the halo exchange), and source rows are fetched with
  batched dma_gathers (windowed for the signed-16-bit index reach).
  Per-partition gather indices are sorted so each DMA queue walks the
  table mostly monotonically.
- Readout: per-graph mean via an indicator-matrix matmul accumulated in
  PSUM, partials AllReduced, then the small MLP head replicated per core.

Host-side work is integer indexing / layout only; all floating-point math
runs on the NeuronCores.
"""

import os
import numpy as np
from contextlib import ExitStack

import concourse.bass as bass
import concourse.tile as tile
from concourse import bacc, mybir
from concourse import bass_utils

AF = mybir.ActivationFunctionType
ALU = mybir.AluOpType
F32 = mybir.dt.float32
I16 = mybir.dt.int16
BF16 = mybir.dt.bfloat16
SPLIT = 32768           # dma_gather int16 index reach

NC = 8
P = 128
NEG_BIG = -1.0e30

LAST_EXEC_NS = None
_CACHE = {}


# ----------------------------------------------------------------------------
# Host-side integer preprocessing
# ----------------------------------------------------------------------------

def _pack_idx16(flat):
    """Pack a flat gather-position list into the dma_gather int16 SBUF
    layout: value for position i sits at [i % 16, i // 16], replicated
    across the 8 groups of 16 partitions."""
    n = len(flat)
    assert n % 16 == 0
    arr = np.asarray(flat, np.int64).reshape(n // 16, 16).T  # [16, n/16]
    return np.tile(arr, (8, 1)).astype(np.int16)


def _preprocess(src, dst, gid, N):
    Ncore = N // NC
    TILES = (Ncore + P - 1) // P
    NT = TILES * P
    NTS = NT + 1                  # shard rows incl. trailing dummy

    deg = np.bincount(dst, minlength=N)

    eorder = np.argsort(dst, kind="stable")
    srcs_sorted = src[eorder].astype(np.int64)
    rowptr = np.zeros(N + 1, np.int64)
    rowptr[1:] = np.cumsum(deg)

    node_order = np.full((NC, NT), -1, np.int64)
    for c in range(NC):
        d = deg[c * Ncore:(c + 1) * Ncore]
        o = np.argsort(-d, kind="stable")
        node_order[c, :Ncore] = c * Ncore + o

    degp = np.zeros((NC, NT), np.int64)
    for c in range(NC):
        real = node_order[c] >= 0
        degp[c, real] = deg[node_order[c][real]]
    K_t = np.maximum(degp.reshape(NC, TILES, P).max(axis=(0, 2)), 1)
    S1 = int(K_t.sum())
    off_t = np.zeros(TILES + 1, np.int64)
    off_t[1:] = np.cumsum(K_t)

    pos2 = np.zeros(N, np.int64)
    for c in range(NC):
        real = node_order[c] >= 0
        pos2[node_order[c][real]] = c * NTS + np.nonzero(real)[0]

    T2ROWS = NC * NTS

    # Overlapping int16-reach windows over the layer-2 table. Every row is
    # inside >= 1 window; rows in overlaps are assigned to balance the
    # per-partition counts (the padding cost is the per-tile max count).
    if T2ROWS <= SPLIT:
        wbase = [0]
    else:
        span = T2ROWS - SPLIT
        wbase = [0, span // 2, span]
    NW = len(wbase)
    # one dummy (el=-inf) row per window: each core's shard ends with one
    dummies = []
    for b in wbase:
        d = None
        for c in range(NC):
            row = c * NTS + NT
            if b <= row < b + SPLIT:
                d = row
                break
        assert d is not None
        dummies.append(d)

    # layer-1 slot sources (per core), -1 = padding slot
    slot_src = np.full((NC, S1, P), -1, np.int64)
    # layer-2 window-assigned slots per (core, tile, partition, window)
    wslots = [[[[[] for _ in range(NW)] for _ in range(P)]
               for _ in range(TILES)] for _ in range(NC)]
    Mmat = np.zeros((NC, P, TILES * P), np.float32)
    scat = np.zeros((NC, P, 1), np.int32)

    for c in range(NC):
        g_lo = gid[c * Ncore]
        assert gid[(c + 1) * Ncore - 1] - g_lo + 1 <= P
        scat[c, :, 0] = g_lo + np.arange(P)
        for t in range(TILES):
            for p in range(P):
                n = node_order[c, t * P + p]
                if n < 0:
                    continue
                dn = deg[n]
                es = srcs_sorted[rowptr[n]:rowptr[n] + dn]
                slot_src[c, off_t[t]:off_t[t] + dn, p] = es
                Mmat[c, p, t * P + (gid[n] - g_lo)] = 1.0
                # balanced window assignment (forced singles first)
                ws = wslots[c][t][p]
                items = []
                for q in pos2[es]:
                    elig = [w for w in range(NW)
                            if wbase[w] <= q < wbase[w] + SPLIT]
                    items.append((len(elig), q, elig))
                items.sort(key=lambda x: x[0])
                for _, q, elig in items:
                    w = min(elig, key=lambda w: len(ws[w]))
                    ws[w].append(q - wbase[w])

    # per-tile per-window padded counts, common across cores
    nW = np.zeros((TILES, NW), np.int64)
    for t in range(TILES):
        for c in range(NC):
            for p in range(P):
                for w in range(NW):
                    nW[t, w] = max(nW[t, w], len(wslots[c][t][p][w]))
    nW[:, 0] = np.maximum(nW[:, 0], 1)
    offW = np.zeros((TILES + 1, NW), np.int64)
    offW[1:] = np.cumsum(nW, axis=0)
    CW = nW.sum(axis=0).astype(np.int64)      # columns per window

    idx2 = [np.zeros((NC, P, max(int(CW[w]), 1) * 8), np.int16)
            for w in range(NW)]
    for c in range(NC):
        for t in range(TILES):
            for w in range(NW):
                nw = int(nW[t, w])
                if nw == 0:
                    continue
                fa = np.full((nw, P), dummies[w] - wbase[w], np.int64)
                for p in range(P):
                    v = sorted(wslots[c][t][p][w])
                    fa[:len(v), p] = v
                idx2[w][c][:, int(offW[t, w]) * 8:int(offW[t + 1, w]) * 8] = \
                    _pack_idx16(fa.reshape(-1))

    GROWS = 640
    cnt = np.maximum(np.bincount(gid, minlength=GROWS).astype(np.float32), 1.0)

    return dict(
        Ncore=Ncore, TILES=TILES, NT=NT, NTS=NTS, K_t=K_t, S1=S1, off_t=off_t,
        T2ROWS=T2ROWS, NW=NW, wbase=wbase, nW=nW, offW=offW, CW=CW,
        GROWS=GROWS, node_order=node_order, slot_src=slot_src,
        idx2=idx2, Mmat=Mmat, scat=scat, cnt=cnt.reshape(GROWS, 1),
    )


# ----------------------------------------------------------------------------
# Bass program
# ----------------------------------------------------------------------------

def _build_program(N, F, Gn, C, pre):
    TILES, NT, NTS = pre["TILES"], pre["NT"], pre["NTS"]
    K_t, S1, off_t = pre["K_t"], pre["S1"], pre["off_t"]
    NW, wbase, nW, offW, CW = (pre["NW"], pre["wbase"], pre["nW"],
                               pre["offW"], pre["CW"])
    T2ROWS, GROWS = pre["T2ROWS"], pre["GROWS"]

    HF = 2 * F                  # 200
    RW = HF + 4                 # W1e row: feat(200) el(2) er(2)
    CW2 = HF + 2                # slot matmul cols: feat(200) el(2)
    BB = F + 1                  # t2-row branch block: feat2(100) el2(1)
    TROW = 384                  # t2 row: 3 x BB + pad (768B, gather-aligned)
    FI = F + 1                  # x rows + pad-indicator row
    XGTOT = int(FI * 3 * P * S1)

    nc = bacc.Bacc("TRN2", target_bir_lowering=False, debug=False,
                   enable_asserts=False, num_devices=NC, num_swdge_queues=4)

    xgt = nc.dram_tensor("xgt", [XGTOT], BF16, kind="ExternalInput")
    xot = nc.dram_tensor("xot", [TILES * FI * 3 * P], BF16,
                         kind="ExternalInput")
    W1 = nc.dram_tensor("W1", [F, HF], F32, kind="ExternalInput")
    al1 = nc.dram_tensor("al1", [2, F], F32, kind="ExternalInput")
    ar1 = nc.dram_tensor("ar1", [2, F], F32, kind="ExternalInput")
    b1 = nc.dram_tensor("b1", [HF], F32, kind="ExternalInput")
    W2 = nc.dram_tensor("W2", [HF, F], F32, kind="ExternalInput")
    al2 = nc.dram_tensor("al2", [1, F], F32, kind="ExternalInput")
    ar2 = nc.dram_tensor("ar2", [1, F], F32, kind="ExternalInput")
    b2 = nc.dram_tensor("b2", [F], F32, kind="ExternalInput")
    Wfc = nc.dram_tensor("Wfc", [3 * F, F], F32, kind="ExternalInput")
    bfc = nc.dram_tensor("bfc", [F], F32, kind="ExternalInput")
    Wcls = nc.dram_tensor("Wcls", [F, C], F32, kind="ExternalInput")
    bcls = nc.dram_tensor("bcls", [C], F32, kind="ExternalInput")
    idx2 = [nc.dram_tensor(f"idx2w{w}", [P, max(int(CW[w]), 1) * 8], I16,
                           kind="ExternalInput") for w in range(NW)]
    Mm = nc.dram_tensor("Mm", [P, TILES * P], BF16, kind="ExternalInput")
    scat = nc.dram_tensor("scat", [P, 1], mybir.dt.int32, kind="ExternalInput")
    cnt = nc.dram_tensor("cnt", [GROWS, 1], F32, kind="ExternalInput")
    wrow = nc.dram_tensor("wrow", [1, RW], F32, kind="ExternalInput")
    out = nc.dram_tensor("out", [Gn, C], F32, kind="ExternalOutput")

    def bcast(handle, n, parts=P):
        ap = handle.ap()
        return bass.AP(tensor=ap.tensor, offset=0, ap=[[0, parts], [1, n]])

    def xgt_tile(t):
        """AP for tile t's expanded-x block: [FI, 3*K_t*P] contiguous."""
        o = int(off_t[t]) * FI * 3 * P
        w = int(K_t[t]) * 3 * P
        return bass.AP(tensor=xgt.ap().tensor, offset=o,
                       ap=[[w, FI], [1, w]])

    with tile.TileContext(nc) as tc, ExitStack() as ctx:
        sing = ctx.enter_context(tc.tile_pool(name="sing", bufs=1))
        xp = ctx.enter_context(tc.tile_pool(name="xp", bufs=2))
        ep = ctx.enter_context(tc.tile_pool(name="ep", bufs=2))
        g2p = ctx.enter_context(tc.tile_pool(name="g2p", bufs=3))
        ixp = ctx.enter_context(tc.tile_pool(name="ixp", bufs=2))
        sm = ctx.enter_context(tc.tile_pool(name="sm", bufs=3))
        hp = ctx.enter_context(tc.tile_pool(name="hp", bufs=2))
        pt1 = ctx.enter_context(tc.tile_pool(name="pt1", bufs=2, space="PSUM"))
        ptp = ctx.enter_context(tc.tile_pool(name="ptp", bufs=1, space="PSUM"))
        pt2 = ctx.enter_context(tc.tile_pool(name="pt2", bufs=2, space="PSUM"))
        pme = ctx.enter_context(tc.tile_pool(name="pme", bufs=1, space="PSUM"))
        dp1 = ctx.enter_context(tc.tile_pool(name="dp1", bufs=1, space="DRAM"))

        ENGS = [nc.sync, nc.scalar, nc.gpsimd]

        # ---------------- constants ----------------
        # W1e: [W1 | W1.al1 | W1.ar1] with a trailing pad-indicator row that
        # pushes padding slots' el/er to -1e30.
        W1e = sing.tile([FI, RW], F32)
        nc.sync.dma_start(out=W1e[0:F, 0:HF], in_=W1[:, :])
        tmp = sing.tile([F, HF], F32)
        attb = sing.tile([F, HF], F32)
        nc.sync.dma_start(out=attb[:], in_=bcast(al1, HF, F))
        nc.vector.tensor_tensor(out=tmp[:], in0=W1e[0:F, 0:HF], in1=attb[:],
                                op=ALU.mult)
        nc.vector.tensor_reduce(out=W1e[0:F, HF:HF + 2],
                                in_=tmp[:].rearrange("p (h f) -> p h f", h=2),
                                axis=mybir.AxisListType.X, op=ALU.add)
        nc.sync.dma_start(out=attb[:], in_=bcast(ar1, HF, F))
        nc.vector.tensor_tensor(out=tmp[:], in0=W1e[0:F, 0:HF], in1=attb[:],
                                op=ALU.mult)
        nc.vector.tensor_reduce(out=W1e[0:F, HF + 2:HF + 4],
                                in_=tmp[:].rearrange("p (h f) -> p h f", h=2),
                                axis=mybir.AxisListType.X, op=ALU.add)
        nc.sync.dma_start(out=W1e[F:FI, :], in_=wrow[:, :])
        # bf16 copy of the extended weight for the slot matmuls
        W1eb = sing.tile([FI, RW], BF16)
        nc.scalar.activation(out=W1eb[:], in_=W1e[:], func=AF.Copy,
                             bias=0.0, scale=1.0)

        W2eb = []
        tmp2 = sing.tile([F, F], F32)
        attb2 = sing.tile([F, F], F32)
        for j in range(2):
            w = sing.tile([F, F + 2], F32, tag=f"W2e{j}", name=f"W2e{j}")
            nc.sync.dma_start(out=w[:, 0:F], in_=W2[j * F:(j + 1) * F, :])
            nc.sync.dma_start(out=attb2[:], in_=bcast(al2, F, F))
            nc.vector.tensor_tensor(out=tmp2[:], in0=w[:, 0:F], in1=attb2[:],
                                    op=ALU.mult)
            nc.vector.tensor_reduce(out=w[:, F:F + 1], in_=tmp2[:],
                                    axis=mybir.AxisListType.X, op=ALU.add)
            nc.sync.dma_start(out=attb2[:], in_=bcast(ar2, F, F))
            nc.vector.tensor_tensor(out=tmp2[:], in0=w[:, 0:F], in1=attb2[:],
                                    op=ALU.mult)
            nc.vector.tensor_reduce(out=w[:, F + 1:F + 2], in_=tmp2[:],
                                    axis=mybir.AxisListType.X, op=ALU.add)
            wb = sing.tile([F, F + 2], BF16, tag=f"W2eb{j}", name=f"W2eb{j}")
            nc.scalar.activation(out=wb[:], in_=w[:], func=AF.Copy,
                                 bias=0.0, scale=1.0)
            W2eb.append(wb)

        # b1 as [F, 2] column pair for the hT-copy bias fold
        b1col = sing.tile([F, 2], F32)
        nc.sync.dma_start(out=b1col[:],
                          in_=bass.AP(tensor=b1.ap().tensor, offset=0,
                                      ap=[[1, F], [F, 2]]))
        b2rep = sing.tile([P, F], F32)
        nc.sync.dma_start(out=b2rep[:], in_=bcast(b2, F))
        bfcrep = sing.tile([P, F], F32)
        nc.sync.dma_start(out=bfcrep[:], in_=bcast(bfc, F))
        bclsrep = sing.tile([P, C], F32)
        nc.sync.dma_start(out=bclsrep[:], in_=bcast(bcls, C))
        wfc_f = sing.tile([F, 3 * F], F32)
        for j in range(3):
            nc.sync.dma_start(out=wfc_f[:, j * F:(j + 1) * F],
                              in_=Wfc[j * F:(j + 1) * F, :])
        wfc_sb = sing.tile([F, 3 * F], BF16)
        nc.scalar.activation(out=wfc_sb[:], in_=wfc_f[:], func=AF.Copy,
                             bias=0.0, scale=1.0)
        wcls_f = sing.tile([F, C], F32)
        nc.sync.dma_start(out=wcls_f[:], in_=Wcls[:, :])
        wcls_sb = sing.tile([F, C], BF16)
        nc.scalar.activation(out=wcls_sb[:], in_=wcls_f[:], func=AF.Copy,
                             bias=0.0, scale=1.0)
        ident = sing.tile([P, P], F32)
        from concourse.masks import make_identity
        make_identity(nc, ident[:])
        identb = sing.tile([P, P], BF16)
        nc.scalar.activation(out=identb[:], in_=ident[:], func=AF.Copy,
                             bias=0.0, scale=1.0)

        scatsb = sing.tile([P, 1], mybir.dt.int32)
        nc.sync.dma_start(out=scatsb[:], in_=scat[:, :])
        drow2 = sing.tile([1, TROW], BF16)
        nc.vector.memset(drow2[:], 0.0)
        for b in range(3):
            nc.vector.memset(drow2[0:1, b * BB + F:b * BB + F + 1], NEG_BIG)
        partial = sing.tile([P, 3 * F], F32)

        # ---------------- layer 1 (tile-major, 3 branches per tile) --------
        # t2 node table rows are bf16: [b0: feat2(100) el2 | b1 | b2 | pad].
        # One gather per edge then serves all three branches.
        t2all = dp1.tile([NTS, TROW], BF16, tag="t2all")
        t2f = dp1.tile([T2ROWS, TROW], BF16, tag="t2full",
                       addr_space="Shared")
        # zero-fill t2all once (covers the pad columns + dummy row)
        zrow = sing.tile([P, TROW], BF16)
        nc.vector.memset(zrow[:], 0.0)
        for j in range(TILES):
            ENGS[j % 3].dma_start(out=t2all[j * P:(j + 1) * P, :],
                                  in_=zrow[:])
        nc.sync.dma_start(out=t2all[NT:NT + 1, :], in_=drow2[:])

        # er table for own (destination) nodes, all branches
        er2tabs = []
        ertabs = []
        for b in range(3):
            ertabs.append(sing.tile([P, 2 * TILES], F32, tag=f"ertab{b}",
                                    name=f"ertab{b}"))
            er2tabs.append(sing.tile([P, TILES], F32, tag=f"er2tab{b}",
                                     name=f"er2tab{b}"))
        for t in range(TILES):
            xoc = xp.tile([FI, 3 * P], BF16, tag="xoc")
            xo_ap = bass.AP(tensor=xot.ap().tensor,
                            offset=t * FI * 3 * P,
                            ap=[[3 * P, FI], [1, 3 * P]])
            ENGS[t % 3].dma_start(out=xoc[:], in_=xo_ap)
            pse = pt2.tile([P, P], F32, tag="pt2")
            for b in range(3):
                nc.tensor.matmul(pse[:, b * 4:b * 4 + 4],
                                 lhsT=xoc[:, b * P:(b + 1) * P],
                                 rhs=W1eb[:, HF:HF + 4], start=(b == 0),
                                 stop=(b == 2), skip_group_check=True)
            for b in range(3):
                nc.scalar.activation(out=ertabs[b][:, 2 * t:2 * t + 2],
                                     in_=pse[:, b * 4 + 2:b * 4 + 4],
                                     func=AF.Copy, bias=0.0, scale=1.0)

        for t in range(TILES):
            K = int(K_t[t])
            for b in range(3):
                # load this branch's expanded-x slice of the tile block,
                # rotating issue across the three DMA-capable engines
                xgc = xp.tile([FI, K * P], BF16, tag="xgc")
                o3 = int(off_t[t]) * FI * 3 * P
                src_ap = bass.AP(tensor=xgt.ap().tensor,
                                 offset=o3 + b * FI * K * P,
                                 ap=[[K * P, FI], [1, K * P]])
                ENGS[(t + b) % 3].dma_start(out=xgc[:], in_=src_ap)
                # slot-major feature tile G: [P, K, CW2]
                G = ep.tile([P, K * CW2], BF16, tag="G1")
                Gv = G[:].rearrange("p (k r) -> p k r", r=CW2)
                for k0 in range(0, K, 4):
                    kw = min(4, K - k0)
                    ps = pt1.tile([P, 1024], F32, tag="pt1")
                    for j in range(kw):
                        nc.tensor.matmul(
                            ps[:, j * 256:j * 256 + CW2],
                            lhsT=xgc[:, (k0 + j) * P:(k0 + j + 1) * P],
                            rhs=W1eb[:, 0:CW2], start=True, stop=True,
                            skip_group_check=True)
                    nc.scalar.activation(
                        out=Gv[:, k0:k0 + kw, :],
                        in_=ps[:].rearrange("p (k r) -> p k r",
                                            r=256)[:, 0:kw, 0:CW2],
                        func=AF.Copy, bias=0.0, scale=1.0)
                # edge softmax batched over the 2 heads
                z_all = sm.tile([P, 2 * K], F32, tag="z")
                zv = z_all[:].rearrange("p (u k) -> p u k", k=K)
                for h in range(2):
                    nc.scalar.activation(
                        out=zv[:, h, :], in_=Gv[:, :, HF + h],
                        func=AF.Identity,
                        bias=ertabs[b][:, 2 * t + h:2 * t + h + 1],
                        scale=1.0)
                nc.vector.scalar_tensor_tensor(
                    out=z_all[:], in0=z_all[:], scalar=0.2, in1=z_all[:],
                    op0=ALU.mult, op1=ALU.max)
                a_all = sm.tile([P, 2 * K], BF16, tag="a")
                nc.scalar.activation(out=a_all[:], in_=z_all[:], func=AF.Exp,
                                     bias=0.0, scale=1.0)
                av = a_all[:].rearrange("p (u k) -> p u k", k=K)
                s_all = sm.tile([P, 2], F32, tag="s")
                nc.vector.tensor_reduce(out=s_all[:], in_=av,
                                        axis=mybir.AxisListType.X, op=ALU.add)
                nc.vector.tensor_scalar_max(out=s_all[:], in0=s_all[:],
                                            scalar1=1e-6)
                rs_all = sm.tile([P, 2], F32, tag="rs")
                nc.vector.reciprocal(out=rs_all[:], in_=s_all[:])
                # weighted aggregation: attention multiplied into G in
                # place, then one strided reduce over slots
                for h in range(2):
                    abc = av[:, h:h + 1, :].rearrange("p o k -> p k o") \
                        .to_broadcast([P, K, F])
                    nc.vector.tensor_tensor(
                        out=Gv[:, :, h * F:(h + 1) * F],
                        in0=Gv[:, :, h * F:(h + 1) * F],
                        in1=abc, op=ALU.mult)
                acc_all = sm.tile([P, 2 * F], BF16, tag="acc")
                accv = acc_all[:].rearrange("p (u f) -> p u f", f=F)
                red = sm.tile([P, HF], F32, tag="red")
                nc.vector.tensor_reduce(
                    out=red[:],
                    in_=Gv.rearrange("p k r -> p r k")[:, 0:HF, :],
                    axis=mybir.AxisListType.X, op=ALU.add)
                nc.vector.tensor_tensor(
                    out=accv[:],
                    in0=red[:].rearrange("p (h f) -> p h f", h=2),
                    in1=rs_all[:].rearrange("p (o u) -> p u o", o=1)
                    .to_broadcast([P, 2, F]),
                    op=ALU.mult)
                # layer-2 table rows for this tile/branch
                hTs = []
                for h in range(2):
                    tp = ptp.tile([P, P], BF16, tag="ptpb")
                    nc.tensor.transpose(tp[0:F, :],
                                        accv[:, h, :], identb[:])
                    hT = hp.tile([F, P], BF16, tag="hT")
                    nc.scalar.activation(out=hT[:], in_=tp[0:F, :],
                                         func=AF.Identity,
                                         bias=b1col[:, h:h + 1], scale=1.0)
                    hTs.append(hT)
                ps2 = pt2.tile([P, F + 2], F32, tag="pt2")
                for j in range(2):
                    nc.tensor.matmul(ps2[:], lhsT=hTs[j][:], rhs=W2eb[j][:],
                                     start=(j == 0), stop=(j == 1),
                                     skip_group_check=True)
                stage = hp.tile([P, BB], BF16, tag="stage")
                nc.scalar.activation(out=stage[:], in_=ps2[:, 0:BB],
                                     func=AF.Copy, bias=0.0, scale=1.0)
                nc.sync.dma_start(
                    out=t2all[t * P:(t + 1) * P, b * BB:(b + 1) * BB],
                    in_=stage[:])
                # own er2 straight from PSUM (avoids a scatter-read later)
                nc.scalar.activation(out=er2tabs[b][:, t:t + 1],
                                     in_=ps2[:, F + 1:F + 2], func=AF.Copy,
                                     bias=0.0, scale=1.0)

        # --- halo exchange: one AllGather of the interleaved table ---
        nc.gpsimd.collective_compute(
            "AllGather", ALU.bypass, replica_groups=[list(range(NC))],
            ins=[t2all[:, :]], outs=[t2f[:, :]])

        # ---------------- layer 2 (all 3 branches per gather) ----------------
        gsem = nc.alloc_semaphore("gather_dma")
        pm = pme.tile([P, 3 * F], F32, tag="pme")
        for t in range(TILES):
            nws = [int(nW[t, w]) for w in range(NW)]
            nk = sum(nws)
            gq = [0]
            G2 = g2p.tile([P, nk * TROW], BF16, tag="G2")
            G2v = G2[:].rearrange("p (k e) -> p k e", e=TROW)
            # dma_gather tops out at 1024 indices per instruction
            g0 = 0
            for w in range(NW):
                if nws[w] == 0:
                    continue
                iw = ixp.tile([P, nws[w] * 8], I16, tag=f"ix{w}")
                nc.sync.dma_start(
                    out=iw[:],
                    in_=idx2[w][:, int(offW[t, w]) * 8:int(offW[t + 1, w]) * 8])
                for c0 in range(0, nws[w], 8):
                    cw = min(8, nws[w] - c0)
                    nc.gpsimd.dma_gather(
                        out_ap=G2v[:, g0 + c0:g0 + c0 + cw, :],
                        in_ap=t2f[wbase[w]:, :] if wbase[w] else t2f[:, :],
                        idxs_ap=iw[:, c0 * 8:(c0 + cw) * 8],
                        num_idxs=cw * P, num_idxs_reg=cw * P,
                        elem_size=TROW, queue_num=gq[0] % 4)
                    gq[0] += 1
                g0 += nws[w]
            Mtt = ixp.tile([P, P], BF16, tag="Mt")
            nc.scalar.dma_start(out=Mtt[:], in_=Mm[:, t * P:(t + 1) * P])
            # batched layer-2 softmax over the 3 branches
            z2 = sm.tile([P, 3 * nk], F32, tag="z2")
            z2v = z2[:].rearrange("p (u k) -> p u k", k=nk)
            for b in range(3):
                nc.scalar.activation(out=z2v[:, b, :],
                                     in_=G2v[:, :, b * BB + F],
                                     func=AF.Identity,
                                     bias=er2tabs[b][:, t:t + 1], scale=1.0)
            nc.vector.scalar_tensor_tensor(
                out=z2[:], in0=z2[:], scalar=0.2, in1=z2[:],
                op0=ALU.mult, op1=ALU.max)
            a2 = sm.tile([P, 3 * nk], BF16, tag="a2")
            nc.scalar.activation(out=a2[:], in_=z2[:], func=AF.Exp,
                                 bias=0.0, scale=1.0)
            a2v = a2[:].rearrange("p (u k) -> p u k", k=nk)
            s2 = sm.tile([P, 3], F32, tag="s2")
            nc.vector.tensor_reduce(out=s2[:], in_=a2v,
                                    axis=mybir.AxisListType.X, op=ALU.add)
            nc.vector.tensor_scalar_max(out=s2[:], in0=s2[:], scalar1=1e-6)
            rs2 = sm.tile([P, 3], F32, tag="rs2")
            nc.vector.reciprocal(out=rs2[:], in_=s2[:])
            acc2 = hp.tile([P, 3 * F], BF16, tag="acc2")
            for b in range(3):
                abc = a2v[:, b:b + 1, :].rearrange("p o k -> p k o") \
                    .to_broadcast([P, nk, F])
                nc.vector.tensor_tensor(
                    out=G2v[:, :, b * BB:b * BB + F],
                    in0=G2v[:, :, b * BB:b * BB + F],
                    in1=abc, op=ALU.mult)
                red2 = sm.tile([P, F], F32, tag="red2")
                nc.vector.tensor_reduce(
                    out=red2[:],
                    in_=G2v.rearrange("p k r -> p r k")[:, b * BB:b * BB + F, :],
                    axis=mybir.AxisListType.X, op=ALU.add)
                nc.vector.scalar_tensor_tensor(
                    out=acc2[:, b * F:(b + 1) * F], in0=red2[:],
                    scalar=rs2[:, b:b + 1], in1=b2rep[:],
                    op0=ALU.mult, op1=ALU.add)
            nc.tensor.matmul(pm[:], lhsT=Mtt[:], rhs=acc2[:],
                             start=(t == 0), stop=(t == TILES - 1),
                             skip_group_check=True)
        nc.scalar.activation(out=partial[:], in_=pm[:], func=AF.Copy,
                             bias=0.0, scale=1.0)

        # ---------------- readout ----------------
        pf = dp1.tile([GROWS, 3 * F], F32, tag="pf")
        rsum = dp1.tile([GROWS, 3 * F], F32, tag="rsum", addr_space="Shared")
        zsb = sing.tile([P, 3 * F], F32)
        nc.vector.memset(zsb[:], 0.0)
        for j in range(GROWS // P):
            nc.sync.dma_start(out=pf[j * P:(j + 1) * P, :], in_=zsb[:])
        nc.gpsimd.indirect_dma_start(
            out=pf[:, :],
            out_offset=bass.IndirectOffsetOnAxis(ap=scatsb[:, 0:1], axis=0),
            in_=partial[:], in_offset=None)
        nc.gpsimd.collective_compute(
            "AllReduce", ALU.add, replica_groups=[list(range(NC))],
            ins=[pf[:, :]], outs=[rsum[:, :]])

        GT = (Gn + P - 1) // P
        for gt in range(GT):
            rt = hp.tile([P, 3 * F], F32, tag="rt")
            nc.sync.dma_start(out=rt[:], in_=rsum[gt * P:(gt + 1) * P, :])
            cntt = sm.tile([P, 1], F32, tag="cntt")
            nc.sync.dma_start(out=cntt[:], in_=cnt[gt * P:(gt + 1) * P, :])
            rc = sm.tile([P, 1], F32, tag="rc")
            nc.vector.reciprocal(out=rc[:], in_=cntt[:, 0:1])
            rbar = hp.tile([P, 3 * F], BF16, tag="rbar")
            nc.scalar.activation(out=rbar[:], in_=rt[:], func=AF.Identity,
                                 bias=0.0, scale=rc[:, 0:1])
            rTs = []
            for j in range(3):
                tp = ptp.tile([P, P], BF16, tag="ptpb")
                nc.tensor.transpose(tp[0:F, :], rbar[:, j * F:(j + 1) * F],
                                    identb[:])
                rT = hp.tile([F, P], BF16, tag=f"rT{j}", name=f"rT{j}")
                nc.scalar.activation(out=rT[:], in_=tp[0:F, :], func=AF.Copy,
                                     bias=0.0, scale=1.0)
                rTs.append(rT)
            psfc = pt2.tile([P, F], F32, tag="pt2")
            for j in range(3):
                nc.tensor.matmul(psfc[:], lhsT=rTs[j][:],
                                 rhs=wfc_sb[:, j * F:(j + 1) * F],
                                 start=(j == 0), stop=(j == 2),
                                 skip_group_check=True)
            tfc = hp.tile([P, F], F32, tag="tfc")
            nc.vector.tensor_tensor(out=tfc[:], in0=psfc[:], in1=bfcrep[:],
                                    op=ALU.add)
            trel = hp.tile([P, F], BF16, tag="trel")
            nc.scalar.activation(out=trel[:], in_=tfc[:], func=AF.Relu,
                                 bias=0.0, scale=1.0)
            tpc = ptp.tile([P, P], BF16, tag="ptpb")
            nc.tensor.transpose(tpc[0:F, :], trel[:], identb[:])
            tT = hp.tile([F, P], BF16, tag="hT2")
            nc.scalar.activation(out=tT[:], in_=tpc[0:F, :], func=AF.Copy,
                                 bias=0.0, scale=1.0)
            pscls = pt2.tile([P, C], F32, tag="pt2")
            nc.tensor.matmul(pscls[:], lhsT=tT[:], rhs=wcls_sb[:],
                             start=True, stop=True)
            ocls = hp.tile([P, C], F32, tag="ocls")
            nc.vector.tensor_tensor(out=ocls[:], in0=pscls[:], in1=bclsrep[:],
                                    op=ALU.add)
            rows = min(P, Gn - gt * P)
            nc.sync.dma_start(out=out[gt * P:gt * P + rows, :],
                              in_=ocls[0:rows, :])

    nc.compile()
    return nc


# ----------------------------------------------------------------------------
# Entry point
# ----------------------------------------------------------------------------

def kernel(**inputs):
    global LAST_EXEC_NS
    xs = [np.ascontiguousarray(np.asarray(inputs[k], np.float32))
          for k in ("x_pkt", "x_arv", "x_stat")]
    src = np.asarray(inputs["src"]).astype(np.int64)
    dst = np.asarray(inputs["dst"]).astype(np.int64)
    gid = np.asarray(inputs["graph_id"]).astype(np.int64)

    N, F = xs[0].shape
    Gn = 500
    C = int(np.asarray(inputs["bcls"]).shape[0])

    pre = _preprocess(src, dst, gid, N)

    key = (N, F, Gn, C, pre["S1"], tuple(pre["CW"].tolist()),
           tuple(pre["K_t"].tolist()), tuple(pre["nW"].reshape(-1).tolist()))
    if key not in _CACHE:
        _CACHE[key] = _build_program(N, F, Gn, C, pre)
    nc = _CACHE[key]

    S1, NT, TILES = pre["S1"], pre["NT"], pre["TILES"]
    K_t, off_t = pre["K_t"], pre["off_t"]
    FI = F + 1

    common = {k: np.ascontiguousarray(np.asarray(inputs[k], np.float32))
              for k in ("W1", "al1", "ar1", "b1", "W2", "al2", "ar2", "b2",
                        "Wfc", "bfc", "Wcls", "bcls")}
    common["cnt"] = pre["cnt"]
    wr = np.zeros((1, 2 * F + 4), np.float32)
    wr[0, 2 * F:] = NEG_BIG
    common["wrow"] = wr

    import ml_dtypes
    BF = ml_dtypes.bfloat16
    xsb = [x.T.astype(BF) for x in xs]
    XGTOT = FI * 3 * P * S1
    in_maps = []
    for c in range(NC):
        m = dict(common)
        # per-tile contiguous expanded-x blocks: [FI, 3, K_t, P]
        xg_flat = np.zeros(XGTOT, BF)
        ss = pre["slot_src"][c]                    # [S1, P]
        for t in range(TILES):
            K = int(K_t[t])
            o = int(off_t[t])
            blk = xg_flat[o * FI * 3 * P:(o + K) * FI * 3 * P] \
                .reshape(3, FI, K, P)
            sst = ss[o:o + K, :]                   # [K, P]
            valid = sst >= 0
            sstc = np.where(valid, sst, 0)
            for b in range(3):
                v = xsb[b][:, sstc.reshape(-1)].reshape(F, K, P)
                v[:, ~valid] = 0
                blk[b, :F] = v
                blk[b, F, ~valid] = 1.0
        m["xgt"] = xg_flat

        no = pre["node_order"][c]
        ov = no >= 0
        noc = np.where(ov, no, 0)
        xo = np.zeros((TILES, FI, 3, P), BF)
        for b in range(3):
            v = xsb[b][:, noc].reshape(F, TILES, P)
            v[:, ~ov.reshape(TILES, P)] = 0
            xo[:, :F, b, :] = v.transpose(1, 0, 2)
        xo[:, F, :, :] = (~ov.reshape(TILES, P))[:, :, None] \
            .transpose(0, 2, 1)
        m["xot"] = xo.reshape(-1)

        for w in range(pre["NW"]):
            m[f"idx2w{w}"] = pre["idx2"][w][c]
        m["Mm"] = pre["Mmat"][c].astype(BF)
        m["scat"] = pre["scat"][c]
        in_maps.append(m)

    trace = os.environ.get("GAT_TRACE", "0") == "1"
    if trace:
        _install_trace_shim()
    r = bass_utils.run_bass_kernel_spmd(nc, in_maps, core_ids=list(range(NC)),
                                        trace=trace)
    LAST_EXEC_NS = r.exec_time_ns
    global TRACE_PATH
    TRACE_PATH = (r.instructions_and_trace[1]
                  if r.instructions_and_trace else None)
    return np.asarray(r.results[0]["out"], np.float32)


def _install_trace_shim():
    import sys, types, contextlib, ctypes
    if "antenv.axon_hooks" in sys.modules:
        return
    so_path = "/opt/axon/libaxon_pjrt.so"
    lib = ctypes.CDLL(so_path)
    if not hasattr(lib, "axon_start_nrt_profile"):
        return
    lib.axon_start_nrt_profile.argtypes = [ctypes.POINTER(ctypes.c_int64),
                                           ctypes.c_size_t]
    lib.axon_start_nrt_profile.restype = ctypes.c_int64
    lib.axon_stop_nrt_profile.argtypes = [ctypes.c_char_p]
    lib.axon_stop_nrt_profile.restype = ctypes.c_int64

    @contextlib.contextmanager
    def _hook(output_dir, device_ids):
        import jax
        jax.devices()
        if device_ids:
            ids = (ctypes.c_int64 * len(device_ids))(*device_ids)
            rc = lib.axon_start_nrt_profile(ids, len(device_ids))
        else:
            rc = lib.axon_start_nrt_profile(None, 0)
        if rc != 0:
            raise RuntimeError(f"axon_start_nrt_profile rc={rc}")
        try:
            yield
        finally:
            n = lib.axon_stop_nrt_profile(str(output_dir).encode())
            print(f"profile: {n} file(s) written to {output_dir}")

    mod = types.ModuleType("antenv.axon_hooks")
    mod.get_axon_ntff_profile_hook = lambda: _hook
    mod.set_axon_ntff_profile_hook = lambda h: None
    sys.modules["antenv.axon_hooks"] = mod
    bass_utils.upload_artifacts = lambda tmpdir: f"file://{tmpdir}"


# revision 20
# speedup vs baseline: 1.0788x; 1.0788x over previous
"""3-branch 2-layer GAT classifier on 8 Trainium2 NeuronCores (Bass/Tile).

Strategy (edge-cut sharding per the hint):
- Nodes (and their incoming edges) are sharded contiguously across the 8
  cores; each core owns N/8 destination nodes for both GAT layers.
- Layer 1 is gather-free: the host expands x rows into per-edge-slot order
  (integer indexing only) as per-tile contiguous blocks holding all three
  branches; the tensor engine computes each slot's [feat | el] row via
  per-slot matmuls against [W | W.al | W.ar]. A trailing indicator row in
  the expanded input turns padding slots' el into -1e30 so they drop out
  of the edge softmax. The big streams are issued round-robin across the
  sync/scalar/gpsimd DMA queues so descriptors spread over the 16 DMA
  engines.
- The edge softmax is batched: one leaky-relu / exp / sum / reciprocal
  instruction per tile covers all 3 branches x 2 heads; exp runs without
  the max trick (logits are O(1)); the weighted aggregation multiplies
  the attention into the slot-feature tile in place and reduces over
  slots with one strided reduce per branch.
- Layer 2: each core computes its shard of the layer-2 node table (rows
  [feat2|el2] per branch, 320 wide) from its aggregated h1 rows, shards
  are AllGathered (the halo exchange), and source rows are fetched with
  batched dma_gathers (windowed for the signed-16-bit index reach).
  Per-partition gather indices are sorted so each DMA queue walks the
  table mostly monotonically.
- Readout: per-graph mean via an indicator-matrix matmul accumulated in
  PSUM, partials AllReduced, then the small MLP head replicated per core.

Host-side work is integer indexing / layout only; all floating-point math
runs on the NeuronCores.
"""

import os
import numpy as np
from contextlib import ExitStack

import concourse.bass as bass
import concourse.tile as tile
from concourse import bacc, mybir
from concourse import bass_utils

AF = mybir.ActivationFunctionType
ALU = mybir.AluOpType
F32 = mybir.dt.float32
I16 = mybir.dt.int16
BF16 = mybir.dt.bfloat16
SPLIT = 32768           # dma_gather int16 index reach

NC = 8
P = 128
NEG_BIG = -1.0e30

LAST_EXEC_NS = None
_CACHE = {}


# ----------------------------------------------------------------------------
# Host-side integer preprocessing
# ----------------------------------------------------------------------------

def _pack_idx16(flat):
    """Pack a flat gather-position list into the dma_gather int16 SBUF
    layout: value for position i sits at [i % 16, i // 16], replicated
    across the 8 groups of 16 partitions."""
    n = len(flat)
    assert n % 16 == 0
    arr = np.asarray(flat, np.int64).reshape(n // 16, 16).T  # [16, n/16]
    return np.tile(arr, (8, 1)).astype(np.int16)


def _preprocess(src, dst, gid, N):
    Ncore = N // NC
    TILES = (Ncore + P - 1) // P
    NT = TILES * P
    NTS = NT + 1                  # shard rows incl. trailing dummy

    deg = np.bincount(dst, minlength=N)

    eorder = np.argsort(dst, kind="stable")
    srcs_sorted = src[eorder].astype(np.int64)
    rowptr = np.zeros(N + 1, np.int64)
    rowptr[1:] = np.cumsum(deg)

    node_order = np.full((NC, NT), -1, np.int64)
    for c in range(NC):
        d = deg[c * Ncore:(c + 1) * Ncore]
        o = np.argsort(-d, kind="stable")
        node_order[c, :Ncore] = c * Ncore + o

    degp = np.zeros((NC, NT), np.int64)
    for c in range(NC):
        real = node_order[c] >= 0
        degp[c, real] = deg[node_order[c][real]]
    K_t = np.maximum(degp.reshape(NC, TILES, P).max(axis=(0, 2)), 1)
    S1 = int(K_t.sum())
    off_t = np.zeros(TILES + 1, np.int64)
    off_t[1:] = np.cumsum(K_t)

    pos2 = np.zeros(N, np.int64)
    for c in range(NC):
        real = node_order[c] >= 0
        pos2[node_order[c][real]] = c * NTS + np.nonzero(real)[0]

    T2ROWS = NC * NTS

    # Overlapping int16-reach windows over the layer-2 table. Every row is
    # inside >= 1 window; rows in overlaps are assigned to balance the
    # per-partition counts (the padding cost is the per-tile max count).
    if T2ROWS <= SPLIT:
        wbase = [0]
    else:
        span = T2ROWS - SPLIT
        wbase = [0, span // 2, span]
    NW = len(wbase)
    # one dummy (el=-inf) row per window: each core's shard ends with one
    dummies = []
    for b in wbase:
        d = None
        for c in range(NC):
            row = c * NTS + NT
            if b <= row < b + SPLIT:
                d = row
                break
        assert d is not None
        dummies.append(d)

    # layer-1 slot sources (per core), -1 = padding slot
    slot_src = np.full((NC, S1, P), -1, np.int64)
    # layer-2 window-assigned slots per (core, tile, partition, window)
    wslots = [[[[[] for _ in range(NW)] for _ in range(P)]
               for _ in range(TILES)] for _ in range(NC)]
    Mmat = np.zeros((NC, P, TILES * P), np.float32)
    scat = np.zeros((NC, P, 1), np.int32)

    for c in range(NC):
        g_lo = gid[c * Ncore]
        assert gid[(c + 1) * Ncore - 1] - g_lo + 1 <= P
        scat[c, :, 0] = g_lo + np.arange(P)
        for t in range(TILES):
            for p in range(P):
                n = node_order[c, t * P + p]
                if n < 0:
                    continue
                dn = deg[n]
                es = srcs_sorted[rowptr[n]:rowptr[n] + dn]
                slot_src[c, off_t[t]:off_t[t] + dn, p] = es
                Mmat[c, p, t * P + (gid[n] - g_lo)] = 1.0
                # balanced window assignment (forced singles first)
                ws = wslots[c][t][p]
                items = []
                for q in pos2[es]:
                    elig = [w for w in range(NW)
                            if wbase[w] <= q < wbase[w] + SPLIT]
                    items.append((len(elig), q, elig))
                items.sort(key=lambda x: x[0])
                for _, q, elig in items:
                    w = min(elig, key=lambda w: len(ws[w]))
                    ws[w].append(q - wbase[w])

    # per-tile per-window padded counts, common across cores
    nW = np.zeros((TILES, NW), np.int64)
    for t in range(TILES):
        for c in range(NC):
            for p in range(P):
                for w in range(NW):
                    nW[t, w] = max(nW[t, w], len(wslots[c][t][p][w]))
    nW[:, 0] = np.maximum(nW[:, 0], 1)
    offW = np.zeros((TILES + 1, NW), np.int64)
    offW[1:] = np.cumsum(nW, axis=0)
    CW = nW.sum(axis=0).astype(np.int64)      # columns per window

    idx2 = [np.zeros((NC, P, max(int(CW[w]), 1) * 8), np.int16)
            for w in range(NW)]
    for c in range(NC):
        for t in range(TILES):
            for w in range(NW):
                nw = int(nW[t, w])
                if nw == 0:
                    continue
                fa = np.full((nw, P), dummies[w] - wbase[w], np.int64)
                for p in range(P):
                    v = sorted(wslots[c][t][p][w])
                    fa[:len(v), p] = v
                idx2[w][c][:, int(offW[t, w]) * 8:int(offW[t + 1, w]) * 8] = \
                    _pack_idx16(fa.reshape(-1))

    GROWS = 640
    cnt = np.maximum(np.bincount(gid, minlength=GROWS).astype(np.float32), 1.0)

    return dict(
        Ncore=Ncore, TILES=TILES, NT=NT, NTS=NTS, K_t=K_t, S1=S1, off_t=off_t,
        T2ROWS=T2ROWS, NW=NW, wbase=wbase, nW=nW, offW=offW, CW=CW,
        GROWS=GROWS, node_order=node_order, slot_src=slot_src,
        idx2=idx2, Mmat=Mmat, scat=scat, cnt=cnt.reshape(GROWS, 1),
    )


# ----------------------------------------------------------------------------
# Bass program
# ----------------------------------------------------------------------------

def _build_program(N, F, Gn, C, pre):
    TILES, NT, NTS = pre["TILES"], pre["NT"], pre["NTS"]
    K_t, S1, off_t = pre["K_t"], pre["S1"], pre["off_t"]
    NW, wbase, nW, offW, CW = (pre["NW"], pre["wbase"], pre["nW"],
                               pre["offW"], pre["CW"])
    T2ROWS, GROWS = pre["T2ROWS"], pre["GROWS"]

    HF = 2 * F                  # 200
    RW = HF + 4                 # W1e row: feat(200) el(2) er(2)
    CW2 = HF + 2                # slot matmul cols: feat(200) el(2)
    BB = F + 1                  # t2-row branch block: feat2(100) el2(1)
    TROW = 384                  # t2 row: 3 x BB + pad (768B, gather-aligned)
    FI = F + 1                  # x rows + pad-indicator row
    XGTOT = int(FI * 3 * P * S1)

    nc = bacc.Bacc("TRN2", target_bir_lowering=False, debug=False,
                   enable_asserts=False, num_devices=NC, num_swdge_queues=4)

    xgt = nc.dram_tensor("xgt", [XGTOT], BF16, kind="ExternalInput")
    xot = nc.dram_tensor("xot", [TILES * FI * 3 * P], BF16,
                         kind="ExternalInput")
    W1 = nc.dram_tensor("W1", [F, HF], F32, kind="ExternalInput")
    al1 = nc.dram_tensor("al1", [2, F], F32, kind="ExternalInput")
    ar1 = nc.dram_tensor("ar1", [2, F], F32, kind="ExternalInput")
    b1 = nc.dram_tensor("b1", [HF], F32, kind="ExternalInput")
    W2 = nc.dram_tensor("W2", [HF, F], F32, kind="ExternalInput")
    al2 = nc.dram_tensor("al2", [1, F], F32, kind="ExternalInput")
    ar2 = nc.dram_tensor("ar2", [1, F], F32, kind="ExternalInput")
    b2 = nc.dram_tensor("b2", [F], F32, kind="ExternalInput")
    Wfc = nc.dram_tensor("Wfc", [3 * F, F], F32, kind="ExternalInput")
    bfc = nc.dram_tensor("bfc", [F], F32, kind="ExternalInput")
    Wcls = nc.dram_tensor("Wcls", [F, C], F32, kind="ExternalInput")
    bcls = nc.dram_tensor("bcls", [C], F32, kind="ExternalInput")
    idx2 = [nc.dram_tensor(f"idx2w{w}", [P, max(int(CW[w]), 1) * 8], I16,
                           kind="ExternalInput") for w in range(NW)]
    Mm = nc.dram_tensor("Mm", [P, TILES * P], BF16, kind="ExternalInput")
    scat = nc.dram_tensor("scat", [P, 1], mybir.dt.int32, kind="ExternalInput")
    cnt = nc.dram_tensor("cnt", [GROWS, 1], F32, kind="ExternalInput")
    wrow = nc.dram_tensor("wrow", [1, RW], F32, kind="ExternalInput")
    out = nc.dram_tensor("out", [Gn, C], F32, kind="ExternalOutput")

    def bcast(handle, n, parts=P):
        ap = handle.ap()
        return bass.AP(tensor=ap.tensor, offset=0, ap=[[0, parts], [1, n]])

    def xgt_tile(t):
        """AP for tile t's expanded-x block: [FI, 3*K_t*P] contiguous."""
        o = int(off_t[t]) * FI * 3 * P
        w = int(K_t[t]) * 3 * P
        return bass.AP(tensor=xgt.ap().tensor, offset=o,
                       ap=[[w, FI], [1, w]])

    with tile.TileContext(nc) as tc, ExitStack() as ctx:
        sing = ctx.enter_context(tc.tile_pool(name="sing", bufs=1))
        xp = ctx.enter_context(tc.tile_pool(name="xp", bufs=2))
        ep = ctx.enter_context(tc.tile_pool(name="ep", bufs=2))
        g2p = ctx.enter_context(tc.tile_pool(name="g2p", bufs=3))
        ixp = ctx.enter_context(tc.tile_pool(name="ixp", bufs=2))
        sm = ctx.enter_context(tc.tile_pool(name="sm", bufs=3))
        hp = ctx.enter_context(tc.tile_pool(name="hp", bufs=2))
        pt1 = ctx.enter_context(tc.tile_pool(name="pt1", bufs=2, space="PSUM"))
        ptp = ctx.enter_context(tc.tile_pool(name="ptp", bufs=1, space="PSUM"))
        pt2 = ctx.enter_context(tc.tile_pool(name="pt2", bufs=2, space="PSUM"))
        pme = ctx.enter_context(tc.tile_pool(name="pme", bufs=1, space="PSUM"))
        dp1 = ctx.enter_context(tc.tile_pool(name="dp1", bufs=1, space="DRAM"))

        ENGS = [nc.sync, nc.scalar, nc.gpsimd]

        # ---------------- constants ----------------
        # W1e: [W1 | W1.al1 | W1.ar1] with a trailing pad-indicator row that
        # pushes padding slots' el/er to -1e30.
        W1e = sing.tile([FI, RW], F32)
        nc.sync.dma_start(out=W1e[0:F, 0:HF], in_=W1[:, :])
        tmp = sing.tile([F, HF], F32)
        attb = sing.tile([F, HF], F32)
        nc.sync.dma_start(out=attb[:], in_=bcast(al1, HF, F))
        nc.vector.tensor_tensor(out=tmp[:], in0=W1e[0:F, 0:HF], in1=attb[:],
                                op=ALU.mult)
        nc.vector.tensor_reduce(out=W1e[0:F, HF:HF + 2],
                                in_=tmp[:].rearrange("p (h f) -> p h f", h=2),
                                axis=mybir.AxisListType.X, op=ALU.add)
        nc.sync.dma_start(out=attb[:], in_=bcast(ar1, HF, F))
        nc.vector.tensor_tensor(out=tmp[:], in0=W1e[0:F, 0:HF], in1=attb[:],
                                op=ALU.mult)
        nc.vector.tensor_reduce(out=W1e[0:F, HF + 2:HF + 4],
                                in_=tmp[:].rearrange("p (h f) -> p h f", h=2),
                                axis=mybir.AxisListType.X, op=ALU.add)
        nc.sync.dma_start(out=W1e[F:FI, :], in_=wrow[:, :])
        # bf16 copy of the extended weight for the slot matmuls
        W1eb = sing.tile([FI, RW], BF16)
        nc.scalar.activation(out=W1eb[:], in_=W1e[:], func=AF.Copy,
                             bias=0.0, scale=1.0)

        W2eb = []
        tmp2 = sing.tile([F, F], F32)
        attb2 = sing.tile([F, F], F32)
        for j in range(2):
            w = sing.tile([F, F + 2], F32, tag=f"W2e{j}", name=f"W2e{j}")
            nc.sync.dma_start(out=w[:, 0:F], in_=W2[j * F:(j + 1) * F, :])
            nc.sync.dma_start(out=attb2[:], in_=bcast(al2, F, F))
            nc.vector.tensor_tensor(out=tmp2[:], in0=w[:, 0:F], in1=attb2[:],
                                    op=ALU.mult)
            nc.vector.tensor_reduce(out=w[:, F:F + 1], in_=tmp2[:],
                                    axis=mybir.AxisListType.X, op=ALU.add)
            nc.sync.dma_start(out=attb2[:], in_=bcast(ar2, F, F))
            nc.vector.tensor_tensor(out=tmp2[:], in0=w[:, 0:F], in1=attb2[:],
                                    op=ALU.mult)
            nc.vector.tensor_reduce(out=w[:, F + 1:F + 2], in_=tmp2[:],
                                    axis=mybir.AxisListType.X, op=ALU.add)
            wb = sing.tile([F, F + 2], BF16, tag=f"W2eb{j}", name=f"W2eb{j}")
            nc.scalar.activation(out=wb[:], in_=w[:], func=AF.Copy,
                                 bias=0.0, scale=1.0)
            W2eb.append(wb)

        # b1 as [F, 2] column pair for the hT-copy bias fold
        b1col = sing.tile([F, 2], F32)
        nc.sync.dma_start(out=b1col[:],
                          in_=bass.AP(tensor=b1.ap().tensor, offset=0,
                                      ap=[[1, F], [F, 2]]))
        b2rep = sing.tile([P, F], F32)
        nc.sync.dma_start(out=b2rep[:], in_=bcast(b2, F))
        bfcrep = sing.tile([P, F], F32)
        nc.sync.dma_start(out=bfcrep[:], in_=bcast(bfc, F))
        bclsrep = sing.tile([P, C], F32)
        nc.sync.dma_start(out=bclsrep[:], in_=bcast(bcls, C))
        wfc_f = sing.tile([F, 3 * F], F32)
        for j in range(3):
            nc.sync.dma_start(out=wfc_f[:, j * F:(j + 1) * F],
                              in_=Wfc[j * F:(j + 1) * F, :])
        wfc_sb = sing.tile([F, 3 * F], BF16)
        nc.scalar.activation(out=wfc_sb[:], in_=wfc_f[:], func=AF.Copy,
                             bias=0.0, scale=1.0)
        wcls_f = sing.tile([F, C], F32)
        nc.sync.dma_start(out=wcls_f[:], in_=Wcls[:, :])
        wcls_sb = sing.tile([F, C], BF16)
        nc.scalar.activation(out=wcls_sb[:], in_=wcls_f[:], func=AF.Copy,
                             bias=0.0, scale=1.0)
        ident = sing.tile([P, P], F32)
        from concourse.masks import make_identity
        make_identity(nc, ident[:])
        identb = sing.tile([P, P], BF16)
        nc.scalar.activation(out=identb[:], in_=ident[:], func=AF.Copy,
                             bias=0.0, scale=1.0)

        scatsb = sing.tile([P, 1], mybir.dt.int32)
        nc.sync.dma_start(out=scatsb[:], in_=scat[:, :])
        drow2 = sing.tile([1, TROW], BF16)
        nc.vector.memset(drow2[:], 0.0)
        for b in range(3):
            nc.vector.memset(drow2[0:1, b * BB + F:b * BB + F + 1], NEG_BIG)
        partial = sing.tile([P, 3 * F], F32)

        # ---------------- layer 1 (tile-major, 3 branches per tile) --------
        # t2 node table rows are bf16: [b0: feat2(100) el2 | b1 | b2 | pad].
        # One gather per edge then serves all three branches.
        t2all = dp1.tile([NTS, TROW], BF16, tag="t2all")
        t2f = dp1.tile([T2ROWS, TROW], BF16, tag="t2full",
                       addr_space="Shared")
        # zero-fill t2all once (covers the pad columns + dummy row)
        zrow = sing.tile([P, TROW], BF16)
        nc.vector.memset(zrow[:], 0.0)
        for j in range(TILES):
            ENGS[j % 3].dma_start(out=t2all[j * P:(j + 1) * P, :],
                                  in_=zrow[:])
        nc.sync.dma_start(out=t2all[NT:NT + 1, :], in_=drow2[:])

        # er table for own (destination) nodes, all branches
        er2tabs = []
        ertabs = []
        for b in range(3):
            ertabs.append(sing.tile([P, 2 * TILES], F32, tag=f"ertab{b}",
                                    name=f"ertab{b}"))
            er2tabs.append(sing.tile([P, TILES], F32, tag=f"er2tab{b}",
                                     name=f"er2tab{b}"))
        for t in range(TILES):
            xoc = xp.tile([FI, 3 * P], BF16, tag="xoc")
            xo_ap = bass.AP(tensor=xot.ap().tensor,
                            offset=t * FI * 3 * P,
                            ap=[[3 * P, FI], [1, 3 * P]])
            nc.gpsimd.dma_start(out=xoc[:], in_=xo_ap)
            pse = pt2.tile([P, P], F32, tag="pt2")
            for b in range(3):
                nc.tensor.matmul(pse[:, b * 4:b * 4 + 4],
                                 lhsT=xoc[:, b * P:(b + 1) * P],
                                 rhs=W1eb[:, HF:HF + 4], start=(b == 0),
                                 stop=(b == 2), skip_group_check=True)
            for b in range(3):
                nc.scalar.activation(out=ertabs[b][:, 2 * t:2 * t + 2],
                                     in_=pse[:, b * 4 + 2:b * 4 + 4],
                                     func=AF.Copy, bias=0.0, scale=1.0)

        for t in range(TILES):
            K = int(K_t[t])
            for b in range(3):
                # load this branch's expanded-x slice of the tile block,
                # rotating issue across the three DMA-capable engines
                xgc = xp.tile([FI, K * P], BF16, tag="xgc")
                o3 = int(off_t[t]) * FI * 3 * P
                src_ap = bass.AP(tensor=xgt.ap().tensor,
                                 offset=o3 + b * FI * K * P,
                                 ap=[[K * P, FI], [1, K * P]])
                nc.gpsimd.dma_start(out=xgc[:], in_=src_ap)
                # slot-major feature tile G: [P, K, CW2]
                G = ep.tile([P, K * CW2], BF16, tag="G1")
                Gv = G[:].rearrange("p (k r) -> p k r", r=CW2)
                for k0 in range(0, K, 4):
                    kw = min(4, K - k0)
                    ps = pt1.tile([P, 1024], F32, tag="pt1")
                    for j in range(kw):
                        nc.tensor.matmul(
                            ps[:, j * 256:j * 256 + CW2],
                            lhsT=xgc[:, (k0 + j) * P:(k0 + j + 1) * P],
                            rhs=W1eb[:, 0:CW2], start=True, stop=True,
                            skip_group_check=True)
                    nc.scalar.activation(
                        out=Gv[:, k0:k0 + kw, :],
                        in_=ps[:].rearrange("p (k r) -> p k r",
                                            r=256)[:, 0:kw, 0:CW2],
                        func=AF.Copy, bias=0.0, scale=1.0)
                # edge softmax batched over the 2 heads
                z_all = sm.tile([P, 2 * K], F32, tag="z")
                zv = z_all[:].rearrange("p (u k) -> p u k", k=K)
                for h in range(2):
                    nc.scalar.activation(
                        out=zv[:, h, :], in_=Gv[:, :, HF + h],
                        func=AF.Identity,
                        bias=ertabs[b][:, 2 * t + h:2 * t + h + 1],
                        scale=1.0)
                nc.vector.scalar_tensor_tensor(
                    out=z_all[:], in0=z_all[:], scalar=0.2, in1=z_all[:],
                    op0=ALU.mult, op1=ALU.max)
                a_all = sm.tile([P, 2 * K], BF16, tag="a")
                nc.scalar.activation(out=a_all[:], in_=z_all[:], func=AF.Exp,
                                     bias=0.0, scale=1.0)
                av = a_all[:].rearrange("p (u k) -> p u k", k=K)
                s_all = sm.tile([P, 2], F32, tag="s")
                nc.vector.tensor_reduce(out=s_all[:], in_=av,
                                        axis=mybir.AxisListType.X, op=ALU.add)
                nc.vector.tensor_scalar_max(out=s_all[:], in0=s_all[:],
                                            scalar1=1e-6)
                rs_all = sm.tile([P, 2], F32, tag="rs")
                nc.vector.reciprocal(out=rs_all[:], in_=s_all[:])
                # weighted aggregation: attention multiplied into G in
                # place, then one strided reduce over slots
                for h in range(2):
                    abc = av[:, h:h + 1, :].rearrange("p o k -> p k o") \
                        .to_broadcast([P, K, F])
                    nc.vector.tensor_tensor(
                        out=Gv[:, :, h * F:(h + 1) * F],
                        in0=Gv[:, :, h * F:(h + 1) * F],
                        in1=abc, op=ALU.mult)
                acc_all = sm.tile([P, 2 * F], BF16, tag="acc")
                accv = acc_all[:].rearrange("p (u f) -> p u f", f=F)
                red = sm.tile([P, HF], F32, tag="red")
                nc.vector.tensor_reduce(
                    out=red[:],
                    in_=Gv.rearrange("p k r -> p r k")[:, 0:HF, :],
                    axis=mybir.AxisListType.X, op=ALU.add)
                nc.vector.tensor_tensor(
                    out=accv[:],
                    in0=red[:].rearrange("p (h f) -> p h f", h=2),
                    in1=rs_all[:].rearrange("p (o u) -> p u o", o=1)
                    .to_broadcast([P, 2, F]),
                    op=ALU.mult)
                # layer-2 table rows for this tile/branch
                hTs = []
                for h in range(2):
                    tp = ptp.tile([P, P], BF16, tag="ptpb")
                    nc.tensor.transpose(tp[0:F, :],
                                        accv[:, h, :], identb[:])
                    hT = hp.tile([F, P], BF16, tag="hT")
                    nc.scalar.activation(out=hT[:], in_=tp[0:F, :],
                                         func=AF.Identity,
                                         bias=b1col[:, h:h + 1], scale=1.0)
                    hTs.append(hT)
                ps2 = pt2.tile([P, F + 2], F32, tag="pt2")
                for j in range(2):
                    nc.tensor.matmul(ps2[:], lhsT=hTs[j][:], rhs=W2eb[j][:],
                                     start=(j == 0), stop=(j == 1),
                                     skip_group_check=True)
                stage = hp.tile([P, BB], BF16, tag="stage")
                nc.scalar.activation(out=stage[:], in_=ps2[:, 0:BB],
                                     func=AF.Copy, bias=0.0, scale=1.0)
                nc.sync.dma_start(
                    out=t2all[t * P:(t + 1) * P, b * BB:(b + 1) * BB],
                    in_=stage[:])
                # own er2 straight from PSUM (avoids a scatter-read later)
                nc.scalar.activation(out=er2tabs[b][:, t:t + 1],
                                     in_=ps2[:, F + 1:F + 2], func=AF.Copy,
                                     bias=0.0, scale=1.0)

        # --- halo exchange: one AllGather of the interleaved table ---
        nc.gpsimd.collective_compute(
            "AllGather", ALU.bypass, replica_groups=[list(range(NC))],
            ins=[t2all[:, :]], outs=[t2f[:, :]])

        # ---------------- layer 2 (all 3 branches per gather) ----------------
        gsem = nc.alloc_semaphore("gather_dma")
        pm = pme.tile([P, 3 * F], F32, tag="pme")
        for t in range(TILES):
            nws = [int(nW[t, w]) for w in range(NW)]
            nk = sum(nws)
            gq = [0]
            G2 = g2p.tile([P, nk * TROW], BF16, tag="G2")
            G2v = G2[:].rearrange("p (k e) -> p k e", e=TROW)
            # dma_gather tops out at 1024 indices per instruction
            g0 = 0
            for w in range(NW):
                if nws[w] == 0:
                    continue
                iw = ixp.tile([P, nws[w] * 8], I16, tag=f"ix{w}")
                nc.sync.dma_start(
                    out=iw[:],
                    in_=idx2[w][:, int(offW[t, w]) * 8:int(offW[t + 1, w]) * 8])
                for c0 in range(0, nws[w], 8):
                    cw = min(8, nws[w] - c0)
                    nc.gpsimd.dma_gather(
                        out_ap=G2v[:, g0 + c0:g0 + c0 + cw, :],
                        in_ap=t2f[wbase[w]:, :] if wbase[w] else t2f[:, :],
                        idxs_ap=iw[:, c0 * 8:(c0 + cw) * 8],
                        num_idxs=cw * P, num_idxs_reg=cw * P,
                        elem_size=TROW, queue_num=gq[0] % 4)
                    gq[0] += 1
                g0 += nws[w]
            Mtt = ixp.tile([P, P], BF16, tag="Mt")
            nc.scalar.dma_start(out=Mtt[:], in_=Mm[:, t * P:(t + 1) * P])
            # batched layer-2 softmax over the 3 branches
            z2 = sm.tile([P, 3 * nk], F32, tag="z2")
            z2v = z2[:].rearrange("p (u k) -> p u k", k=nk)
            for b in range(3):
                nc.scalar.activation(out=z2v[:, b, :],
                                     in_=G2v[:, :, b * BB + F],
                                     func=AF.Identity,
                                     bias=er2tabs[b][:, t:t + 1], scale=1.0)
            nc.vector.scalar_tensor_tensor(
                out=z2[:], in0=z2[:], scalar=0.2, in1=z2[:],
                op0=ALU.mult, op1=ALU.max)
            a2 = sm.tile([P, 3 * nk], BF16, tag="a2")
            nc.scalar.activation(out=a2[:], in_=z2[:], func=AF.Exp,
                                 bias=0.0, scale=1.0)
            a2v = a2[:].rearrange("p (u k) -> p u k", k=nk)
            s2 = sm.tile([P, 3], F32, tag="s2")
            nc.vector.tensor_reduce(out=s2[:], in_=a2v,
                                    axis=mybir.AxisListType.X, op=ALU.add)
            nc.vector.tensor_scalar_max(out=s2[:], in0=s2[:], scalar1=1e-6)
            rs2 = sm.tile([P, 3], F32, tag="rs2")
            nc.vector.reciprocal(out=rs2[:], in_=s2[:])
            acc2 = hp.tile([P, 3 * F], BF16, tag="acc2")
            for b in range(3):
                abc = a2v[:, b:b + 1, :].rearrange("p o k -> p k o") \
                    .to_broadcast([P, nk, F])
                nc.vector.tensor_tensor(
                    out=G2v[:, :, b * BB:b * BB + F],
                    in0=G2v[:, :, b * BB:b * BB + F],
                    in1=abc, op=ALU.mult)
                red2 = sm.tile([P, F], F32, tag="red2")
                nc.vector.tensor_reduce(
                    out=red2[:],
                    in_=G2v.rearrange("p k r -> p r k")[:, b * BB:b * BB + F, :],
                    axis=mybir.AxisListType.X, op=ALU.add)
                nc.vector.scalar_tensor_tensor(
                    out=acc2[:, b * F:(b + 1) * F], in0=red2[:],
                    scalar=rs2[:, b:b + 1], in1=b2rep[:],
                    op0=ALU.mult, op1=ALU.add)
            nc.tensor.matmul(pm[:], lhsT=Mtt[:], rhs=acc2[:],
                             start=(t == 0), stop=(t == TILES - 1),
                             skip_group_check=True)
        nc.scalar.activation(out=partial[:], in_=pm[:], func=AF.Copy,
                             bias=0.0, scale=1.0)

        # ---------------- readout ----------------
        pf = dp1.tile([GROWS, 3 * F], F32, tag="pf")
        rsum = dp1.tile([GROWS, 3 * F], F32, tag="rsum", addr_space="Shared")
        zsb = sing.tile([P, 3 * F], F32)
        nc.vector.memset(zsb[:], 0.0)
        for j in range(GROWS // P):
            nc.sync.dma_start(out=pf[j * P:(j + 1) * P, :], in_=zsb[:])
        nc.gpsimd.indirect_dma_start(
            out=pf[:, :],
            out_offset=bass.IndirectOffsetOnAxis(ap=scatsb[:, 0:1], axis=0),
            in_=partial[:], in_offset=None)
        nc.gpsimd.collective_compute(
            "AllReduce", ALU.add, replica_groups=[list(range(NC))],
            ins=[pf[:, :]], outs=[rsum[:, :]])

        GT = (Gn + P - 1) // P
        for gt in range(GT):
            rt = hp.tile([P, 3 * F], F32, tag="rt")
            nc.sync.dma_start(out=rt[:], in_=rsum[gt * P:(gt + 1) * P, :])
            cntt = sm.tile([P, 1], F32, tag="cntt")
            nc.sync.dma_start(out=cntt[:], in_=cnt[gt * P:(gt + 1) * P, :])
            rc = sm.tile([P, 1], F32, tag="rc")
            nc.vector.reciprocal(out=rc[:], in_=cntt[:, 0:1])
            rbar = hp.tile([P, 3 * F], BF16, tag="rbar")
            nc.scalar.activation(out=rbar[:], in_=rt[:], func=AF.Identity,
                                 bias=0.0, scale=rc[:, 0:1])
            rTs = []
            for j in range(3):
                tp = ptp.tile([P, P], BF16, tag="ptpb")
                nc.tensor.transpose(tp[0:F, :], rbar[:, j * F:(j + 1) * F],
                                    identb[:])
                rT = hp.tile([F, P], BF16, tag=f"rT{j}", name=f"rT{j}")
                nc.scalar.activation(out=rT[:], in_=tp[0:F, :], func=AF.Copy,
                                     bias=0.0, scale=1.0)
                rTs.append(rT)
            psfc = pt2.tile([P, F], F32, tag="pt2")
            for j in range(3):
                nc.tensor.matmul(psfc[:], lhsT=rTs[j][:],
                                 rhs=wfc_sb[:, j * F:(j + 1) * F],
                                 start=(j == 0), stop=(j == 2),
                                 skip_group_check=True)
            tfc = hp.tile([P, F], F32, tag="tfc")
            nc.vector.tensor_tensor(out=tfc[:], in0=psfc[:], in1=bfcrep[:],
                                    op=ALU.add)
            trel = hp.tile([P, F], BF16, tag="trel")
            nc.scalar.activation(out=trel[:], in_=tfc[:], func=AF.Relu,
                                 bias=0.0, scale=1.0)
            tpc = ptp.tile([P, P], BF16, tag="ptpb")
            nc.tensor.transpose(tpc[0:F, :], trel[:], identb[:])
            tT = hp.tile([F, P], BF16, tag="hT2")
            nc.scalar.activation(out=tT[:], in_=tpc[0:F, :], func=AF.Copy,
                                 bias=0.0, scale=1.0)
            pscls = pt2.tile([P, C], F32, tag="pt2")
            nc.tensor.matmul(pscls[:], lhsT=tT[:], rhs=wcls_sb[:],
                             start=True, stop=True)
            ocls = hp.tile([P, C], F32, tag="ocls")
            nc.vector.tensor_tensor(out=ocls[:], in0=pscls[:], in1=bclsrep[:],
                                    op=ALU.add)
            rows = min(P, Gn - gt * P)
            nc.sync.dma_start(out=out[gt * P:gt * P + rows, :],
                              in_=ocls[0:rows, :])

    nc.compile()
    return nc


# ----------------------------------------------------------------------------
# Entry point
# ----------------------------------------------------------------------------

def kernel(**inputs):
    global LAST_EXEC_NS
    xs = [np.ascontiguousarray(np.asarray(inputs[k], np.float32))
          for k in ("x_pkt", "x_arv", "x_stat")]
    src = np.asarray(inputs["src"]).astype(np.int64)
    dst = np.asarray(inputs["dst"]).astype(np.int64)
    gid = np.asarray(inputs["graph_id"]).astype(np.int64)

    N, F = xs[0].shape
    Gn = 500
    C = int(np.asarray(inputs["bcls"]).shape[0])

    pre = _preprocess(src, dst, gid, N)

    key = (N, F, Gn, C, pre["S1"], tuple(pre["CW"].tolist()),
           tuple(pre["K_t"].tolist()), tuple(pre["nW"].reshape(-1).tolist()))
    if key not in _CACHE:
        _CACHE[key] = _build_program(N, F, Gn, C, pre)
    nc = _CACHE[key]

    S1, NT, TILES = pre["S1"], pre["NT"], pre["TILES"]
    K_t, off_t = pre["K_t"], pre["off_t"]
    FI = F + 1

    common = {k: np.ascontiguousarray(np.asarray(inputs[k], np.float32))
              for k in ("W1", "al1", "ar1", "b1", "W2", "al2", "ar2", "b2",
                        "Wfc", "bfc", "Wcls", "bcls")}
    common["cnt"] = pre["cnt"]
    wr = np.zeros((1, 2 * F + 4), np.float32)
    wr[0, 2 * F:] = NEG_BIG
    common["wrow"] = wr

    import ml_dtypes
    BF = ml_dtypes.bfloat16
    xsb = [x.T.astype(BF) for x in xs]
    XGTOT = FI * 3 * P * S1
    in_maps = []
    for c in range(NC):
        m = dict(common)
        # per-tile contiguous expanded-x blocks: [FI, 3, K_t, P]
        xg_flat = np.zeros(XGTOT, BF)
        ss = pre["slot_src"][c]                    # [S1, P]
        for t in range(TILES):
            K = int(K_t[t])
            o = int(off_t[t])
            blk = xg_flat[o * FI * 3 * P:(o + K) * FI * 3 * P] \
                .reshape(3, FI, K, P)
            sst = ss[o:o + K, :]                   # [K, P]
            valid = sst >= 0
            sstc = np.where(valid, sst, 0)
            for b in range(3):
                v = xsb[b][:, sstc.reshape(-1)].reshape(F, K, P)
                v[:, ~valid] = 0
                blk[b, :F] = v
                blk[b, F, ~valid] = 1.0
        m["xgt"] = xg_flat

        no = pre["node_order"][c]
        ov = no >= 0
        noc = np.where(ov, no, 0)
        xo = np.zeros((TILES, FI, 3, P), BF)
        for b in range(3):
            v = xsb[b][:, noc].reshape(F, TILES, P)
            v[:, ~ov.reshape(TILES, P)] = 0
            xo[:, :F, b, :] = v.transpose(1, 0, 2)
        xo[:, F, :, :] = (~ov.reshape(TILES, P))[:, :, None] \
            .transpose(0, 2, 1)
        m["xot"] = xo.reshape(-1)

        for w in range(pre["NW"]):
            m[f"idx2w{w}"] = pre["idx2"][w][c]
        m["Mm"] = pre["Mmat"][c].astype(BF)
        m["scat"] = pre["scat"][c]
        in_maps.append(m)

    trace = os.environ.get("GAT_TRACE", "0") == "1"
    if trace:
        _install_trace_shim()
    r = bass_utils.run_bass_kernel_spmd(nc, in_maps, core_ids=list(range(NC)),
                                        trace=trace)
    LAST_EXEC_NS = r.exec_time_ns
    global TRACE_PATH
    TRACE_PATH = (r.instructions_and_trace[1]
                  if r.instructions_and_trace else None)
    return np.asarray(r.results[0]["out"], np.float32)


def _install_trace_shim():
    import sys, types, contextlib, ctypes
    if "antenv.axon_hooks" in sys.modules:
        return
    so_path = "/opt/axon/libaxon_pjrt.so"
    lib = ctypes.CDLL(so_path)
    if not hasattr(lib, "axon_start_nrt_profile"):
        return
    lib.axon_start_nrt_profile.argtypes = [ctypes.POINTER(ctypes.c_int64),
                                           ctypes.c_size_t]
    lib.axon_start_nrt_profile.restype = ctypes.c_int64
    lib.axon_stop_nrt_profile.argtypes = [ctypes.c_char_p]
    lib.axon_stop_nrt_profile.restype = ctypes.c_int64

    @contextlib.contextmanager
    def _hook(output_dir, device_ids):
        import jax
        jax.devices()
        if device_ids:
            ids = (ctypes.c_int64 * len(device_ids))(*device_ids)
            rc = lib.axon_start_nrt_profile(ids, len(device_ids))
        else:
            rc = lib.axon_start_nrt_profile(None, 0)
        if rc != 0:
            raise RuntimeError(f"axon_start_nrt_profile rc={rc}")
        try:
            yield
        finally:
            n = lib.axon_stop_nrt_profile(str(output_dir).encode())
            print(f"profile: {n} file(s) written to {output_dir}")

    mod = types.ModuleType("antenv.axon_hooks")
    mod.get_axon_ntff_profile_hook = lambda: _hook
    mod.set_axon_ntff_profile_hook = lambda h: None
    sys.modules["antenv.axon_hooks"] = mod
    bass_utils.upload_artifacts = lambda tmpdir: f"file://{tmpdir}"
